# revision 23
# baseline (speedup 1.0000x reference)
"""AudioSNN Trainium2 kernel (v4).

Two-layer leaky-integrate-and-fire SNN (snntorch Leaky, reset-by-subtract),
T=500 recurrent steps over batch 4096, data-parallel over 8 NeuronCores
(512 batch elements per core).

Math (per step t, reference):
    cur1 = x_t @ W1.T + b1
    m1   = beta*m1 + cur1 - H(m1_prev - 1)
    spk1 = H(m1 - 1)
    cur2 = spk1 @ W2.T + b2
    m2   = beta*m2 + cur2 - spk2[t-1]
    spk2 = H(m2 - 1)    -> output [T, B, 5]

Device formulation (per core; recurrent state fp32, matmuls fp16):
  cur1 is spike-exact (host-validated) via a 3-term fp16 hi/lo split packed
  into ONE K=122 matmul per batch half:
    x' = 16*x = xh + xl (fp16), w' = 16*W1.T = wh + wl (fp16)
    lhsT = [wh; bias_hi; wl; bias_lo; wh]  (122 x 128)
    rhs  = [xh; 16s;     xh; 16s;     xl]  -> p1 = K1*cur1', K1 = 256
  Layer-1 state z1 = K1*(m1 - 1).  Batch halves A (0:256) / B (256:512) run
  as independent recurrences in separate PSUM banks:
    p1_h += mask_h @ s_h[t-1]      (A: -0.5*K1*I @ sign; B: -K1*I @ spike)
    z1_h[t] = beta*z1_h[t-1] + p1_h        (DVE)
    s_A[t] = Sign(z1_A[t]) on ACT (+-1 fp16); s_B[t] = z1_B > 0 on DVE (0/1)
  Layer 2 column-grouped ([5,512] -> [128,128], batch quarter q at
  partitions 32q+o) and scaled by S2=64 (fp16-normal-range W2 split).
  y2 = S2*(m2 - 1 - p_q); quarters 0,1 consume sign-form (w2a = S2*0.5*W2.T,
  p from C2a = 0.5*sum(W2)+b2+beta-1), quarters 2,3 spike-form
  (w2b = S2*W2.T, C2b = b2+beta-1):
    p2[32q:32q+5,:] = w2{a,b}hi^T.T @ s_q + w2{a,b}lo^T.T @ s_q
    p2 += (-S2*Imask)_fp16 @ spk2'[t-1]
    y2[t] = beta*y2[t-1] + p2              (DVE)
    spk2'[t] = (y2[t] > -S2*p')            (DVE, fp16 0/1, DMA'd per chunk)
"""

import sys

sys.path.insert(0, "/opt/trn_rl_repo")

from contextlib import ExitStack

import numpy as np

from concourse import bacc, mybir, tile
from concourse.bass_utils import run_bass_kernel_spmd

BETA = 0.9
T, F, H, O = 500, 40, 128, 5
NCORES = 8
BC = 512  # batch per core
G2 = 256  # batch per half (layer-1 chain granularity)
CH = 20  # time steps per DMA chunk (must divide T)
S2 = 64.0  # layer-2 state scale
XS = 16.0  # x scale; W1 also scaled by XS; layer-1 state scale K1 = XS*XS
AHEAD = 2  # cur1 matmul emission lead (steps)
F32 = mybir.dt.float32
FP16 = mybir.dt.float16

MULT = mybir.AluOpType.mult
ADD = mybir.AluOpType.add
IS_GT = mybir.AluOpType.is_gt

KC = 3 * F + 2  # 122: [xh(40); ones; xh(40); ones; xl(40)]


def build(nc, n_steps=T, ch=CH):
    n_chunks = n_steps // ch

    x_d = nc.dram_tensor(
        "x_cat", [n_chunks, KC * ch * BC], FP16, kind="ExternalInput"
    ).ap()
    w1a_d = nc.dram_tensor("w1cat_a", [KC, H], FP16, kind="ExternalInput").ap()
    w1b_d = nc.dram_tensor("w1cat_b", [KC, H], FP16, kind="ExternalInput").ap()
    mska_d = nc.dram_tensor("mask_a", [H, H], FP16, kind="ExternalInput").ap()
    nim_d = nc.dram_tensor("neg_i_mask", [H, H], FP16, kind="ExternalInput").ap()
    w2_d = [
        nc.dram_tensor(nm, [H, O], FP16, kind="ExternalInput").ap()
        for nm in ["w2ahi", "w2alo", "w2bhi", "w2blo"]
    ]
    nth_d = nc.dram_tensor("nthr2", [H, 1], F32, kind="ExternalInput").ap()
    z2i_d = nc.dram_tensor("z2init", [H, H], F32, kind="ExternalInput").ap()
    out_d = nc.dram_tensor(
        "out", [n_chunks, 4 * O * ch * H], FP16, kind="ExternalOutput"
    ).ap()

    with tile.TileContext(nc) as tc, ExitStack() as ctx:
        const = ctx.enter_context(tc.tile_pool(name="const", bufs=1))
        state = ctx.enter_context(tc.tile_pool(name="state", bufs=1))
        xin = ctx.enter_context(tc.tile_pool(name="xin", bufs=3))
        outp = ctx.enter_context(tc.tile_pool(name="outp", bufs=3))
        ps1 = ctx.enter_context(tc.tile_pool(name="ps1", bufs=3, space="PSUM"))
        ps2 = ctx.enter_context(tc.tile_pool(name="ps2", bufs=2, space="PSUM"))

        w1a_s = const.tile([KC, H], FP16, tag="w1a")
        w1b_s = const.tile([KC, H], FP16, tag="w1b")
        mska_s = const.tile([H, H], FP16, tag="mska")
        nim_s = const.tile([H, H], FP16, tag="nim")
        w2_s = [
            const.tile([H, O], FP16, tag=f"w2_{i}", name=f"w2_{i}")
            for i in range(4)
        ]
        nth_s = const.tile([H, 1], F32, tag="nth")
        for s, d in [
            (w1a_s, w1a_d),
            (w1b_s, w1b_d),
            (mska_s, mska_d),
            (nim_s, nim_d),
            (nth_s, nth_d),
        ] + list(zip(w2_s, w2_d)):
            nc.sync.dma_start(out=s[:], in_=d[:])

        # Recurrent state, ping-pong buffered (index = t % 2).
        z1 = [state.tile([H, BC], F32, tag=f"z1_{p}", name=f"z1_{p}") for p in range(2)]
        g = [state.tile([H, BC], FP16, tag=f"g_{p}", name=f"g_{p}") for p in range(2)]
        z2 = [state.tile([H, H], F32, tag=f"z2_{p}", name=f"z2_{p}") for p in range(2)]
        spk0 = state.tile([H, H], FP16, tag="spk0")

        nc.vector.memset(z1[1][:], -XS * XS)  # m1(0)=0 -> z1 = -K1
        nc.vector.memset(g[1][:, 0:G2], -1.0)  # sign(-z)
        nc.vector.memset(g[1][:, G2:BC], 0.0)  # spike form
        nc.sync.dma_start(out=z2[1][:], in_=z2i_d[:])
        nc.vector.memset(spk0[:], -1.0)  # sign form: no spike

        xts = {}
        p1s = {}
        ot = None
        spk_prev = spk0[:]

        def fetch_x(chk):
            """Issue the chunk-chk x load, split across the three DMA
            issue paths (SWDGE + both HWDGE rings) so it streams at
            ~100 GB/s instead of a single ring's ~27."""
            xt = xin.tile([KC, ch * BC], FP16, tag="xt")
            rsp = [0, 70, 101, KC]
            for eng, (r0, r1) in zip(
                (nc.gpsimd, nc.scalar, nc.sync), zip(rsp[:-1], rsp[1:])
            ):
                eng.dma_start(
                    out=xt[r0:r1, :],
                    in_=x_d[chk : chk + 1, r0 * ch * BC : r1 * ch * BC],
                )
            xts[chk] = xt
            xts.pop(chk - 3, None)

        def emit_c1(tf):
            """Emit the cur1 matmuls for step tf (one per batch half)."""
            chk, st = divmod(tf, ch)
            xt = xts[chk]
            pair = []
            for hf in range(2):
                p1 = ps1.tile([H, G2], F32, tag=f"p1_{hf}")
                nc.tensor.matmul(
                    p1[:],
                    w1a_s[:] if hf == 0 else w1b_s[:],
                    xt[:, st * BC + hf * G2 : st * BC + (hf + 1) * G2],
                    start=True,
                    stop=False,
                )
                pair.append(p1)
            p1s[tf] = pair

        l2state = {"spk_prev": spk_prev, "ot": None}

        def emit_l2(tl):
            """Column-grouped layer 2 for step tl ([128,128], scale S2)."""
            chk, st = divmod(tl, ch)
            cur, prv = tl % 2, 1 - (tl % 2)
            if st == 0:
                l2state["ot"] = outp.tile([H, ch * H], FP16, tag="ot", name="ot")
            ot = l2state["ot"]
            p2 = ps2.tile([H, H], F32, tag="p2")
            nc.tensor.matmul(
                p2[:], nim_s[:], l2state["spk_prev"], start=True, stop=False
            )
            order = [(0, 0), (0, 1), (1, 0), (1, 1), (2, 2), (2, 3), (3, 2), (3, 3)]
            for i, (w, q) in enumerate(order):
                nc.tensor.matmul(
                    p2[32 * q : 32 * q + O, :],
                    w2_s[w],
                    g[cur][:, q * H : (q + 1) * H],
                    start=False,
                    stop=(i == 7),
                    tile_position=(0, 32 * q),
                )
            nc.vector.scalar_tensor_tensor(
                z2[cur][:], z2[prv][:], BETA, p2[:], MULT, ADD
            )
            o_slice = ot[:, st * H : (st + 1) * H]
            nc.scalar.sign(o_slice, z2[cur][:], bias=nth_s[:])
            l2state["spk_prev"] = o_slice
            if st == ch // 2 - 1 or st == ch - 1:
                lo_st = 0 if st == ch // 2 - 1 else ch // 2
                for q in range(4):
                    nc.sync.dma_start(
                        out=out_d[
                            chk : chk + 1,
                            q * O * ch * H + lo_st * O * H
                            : q * O * ch * H + (st + 1) * O * H,
                        ],
                        in_=ot[32 * q : 32 * q + O, lo_st * H : (st + 1) * H],
                    )

        fetch_x(0)
        for tf in range(min(AHEAD, n_steps)):
            emit_c1(tf)

        for t in range(n_steps):
            chk, st = divmod(t, ch)
            if st == 1 and chk + 1 < n_chunks:
                fetch_x(chk + 1)  # prefetch a full chunk ahead
            cur, prv = t % 2, 1 - (t % 2)

            # ---- layer 1, independent batch halves ----
            for hf, msk in ((0, mska_s), (1, mska_s)):
                bsl = slice(hf * G2, (hf + 1) * G2)
                p1 = p1s[t][hf]
                nc.tensor.matmul(
                    p1[:], msk[:], g[prv][:, bsl], start=False, stop=True
                )
                nc.vector.scalar_tensor_tensor(
                    z1[cur][:, bsl], z1[prv][:, bsl], BETA, p1[:], MULT, ADD
                )
                if hf == 0:
                    nc.scalar.sign(g[cur][:, bsl], z1[cur][:, bsl])
                else:
                    # {0,2}-valued spikes: same -0.5*K1*I fold mask as the
                    # sign-form half (w2 for these quarters is halved)
                    nc.vector.tensor_scalar(
                        g[cur][:, bsl], z1[cur][:, bsl], 0.0, 2.0, IS_GT, MULT
                    )
            del p1s[t]
            # emit the step-(t+AHEAD) cur1 matmuls HERE: they sit between the
            # folds and the rest of the PE FIFO, streaming while this step's
            # spikes are still in flight
            if t + AHEAD < n_steps:
                emit_c1(t + AHEAD)
            # layer 2 runs one step BEHIND layer 1: every op in emit_l2(t-1)
            # has its inputs ready, so neither the PE nor the DVE FIFO ever
            # blocks the layer-1 chain on layer-2 work
            if t > 0:
                emit_l2(t - 1)
        emit_l2(n_steps - 1)


def _split16(a):
    hi = a.astype(np.float16)
    lo = (a - hi.astype(np.float32)).astype(np.float16)
    return hi, lo


def host_inputs(x, W1, b1, W2, b2, n_steps=T, ch=CH):
    """Shard + precompute all per-core device input arrays."""
    n_chunks = n_steps // ch
    x = np.asarray(x, np.float32)[:, :n_steps, :]
    W1 = np.asarray(W1, np.float32)
    b1 = np.asarray(b1, np.float32)
    W2 = np.asarray(W2, np.float32)
    b2 = np.asarray(b2, np.float32)

    # x*XS split into fp16 hi+lo; rows: [xh(40); 16s; xh(40); 16s; xl(40)]
    xs = x.reshape(NCORES, BC, n_steps, F).transpose(0, 2, 3, 1)  # [8,T',40,512]
    xh = (XS * xs).astype(np.float16)
    xl = (XS * xs - xh.astype(np.float32)).astype(np.float16)
    cat = np.empty((NCORES, n_steps, KC, BC), np.float16)
    cat[:, :, 0:F, :] = xh
    cat[:, :, F, :] = np.float16(XS)
    cat[:, :, F + 1 : 2 * F + 1, :] = xh
    cat[:, :, 2 * F + 1, :] = np.float16(XS)
    cat[:, :, 2 * F + 2 :, :] = xl
    cat = cat.reshape(NCORES, n_chunks, ch, KC, BC).transpose(0, 1, 3, 2, 4)
    x_cat = np.ascontiguousarray(cat).reshape(NCORES, n_chunks, KC * ch * BC)

    # lhsT rows: [wh(40); bias_hi; wl(40); bias_lo; wh(40)]
    # half A (sign-form reset): bias b1 + beta - 1.5; half B (spike-form
    # reset): bias b1 + beta - 1.0
    w1t = W1.T * XS  # [40, 128]
    w1hi, w1lo = _split16(w1t)

    def w1cat_for(bias_shift):
        b1p = (b1 + BETA - bias_shift) * XS
        bhi, blo = _split16(b1p[None, :])
        return np.concatenate([w1hi, bhi, w1lo, blo, w1hi], axis=0)  # [122,128]

    w1cat_a = w1cat_for(1.5)
    w1cat_b = w1cat_for(1.0)

    K1 = XS * XS
    mask_a = (-0.5 * K1 * np.eye(H)).astype(np.float16)  # both halves
    used = np.zeros(H, np.float32)
    for q in range(4):
        used[32 * q : 32 * q + O] = 1.0
    # output spikes are in sign form: -S2*spk = -0.5*S2*s - 0.5*S2, the
    # constant part is absorbed into a state shift of 0.5*S2/(1-beta)
    neg_i_mask = (-0.5 * S2 * np.diag(used)).astype(np.float16)

    w2a = (S2 * 0.5 * W2.T).astype(np.float32)  # sign-form quarters
    w2b = (S2 * 0.5 * W2.T).astype(np.float32)  # {0,2}-spike quarters
    w2ahi, w2alo = _split16(w2a)
    w2bhi, w2blo = _split16(w2b)

    C2a = 0.5 * W2.sum(axis=1) + b2 + BETA - 1.0
    C2b = b2 + BETA - 1.0
    pa = (C2a / (1.0 - BETA)).astype(np.float32)
    pb = (C2b / (1.0 - BETA)).astype(np.float32)
    shift = 0.5 * S2 / (1.0 - BETA)  # 320
    nthr2 = np.zeros((H, 1), np.float32)
    z2init = np.zeros((H, H), np.float32)
    for q in range(4):
        p = pa if q < 2 else pb
        nthr2[32 * q : 32 * q + O, 0] = -(shift - S2 * p)
        z2init[32 * q : 32 * q + O, :] = (S2 * (-1.0 - p) + shift)[:, None]

    shared = {
        "w1cat_a": w1cat_a,
        "w1cat_b": w1cat_b,
        "mask_a": mask_a,
        "neg_i_mask": neg_i_mask,
        "w2ahi": w2ahi,
        "w2alo": w2alo,
        "w2bhi": w2bhi,
        "w2blo": w2blo,
        "nthr2": nthr2,
        "z2init": z2init,
    }
    return [{"x_cat": x_cat[c], **shared} for c in range(NCORES)]


def assemble(results, n_steps=T, ch=CH):
    """per-core out [n_chunks, 4*5*ch*128] fp16 -> [T', B, O] float32."""
    n_chunks = n_steps // ch
    outs = []
    for r in results:
        a = np.asarray(r["out"]).astype(np.float32)
        a = (a + 1.0) * 0.5  # sign form -> 0/1
        # two half-chunk DMAs per quarter: [k, q, half, o, st2, b]
        a = a.reshape(n_chunks, 4, 2, O, ch // 2, H)
        a = a.transpose(0, 2, 4, 1, 5, 3).reshape(n_steps, BC, O)
        outs.append(a)
    return np.concatenate(outs, axis=1)


LAST_RESULT = None  # BassKernelResults of the most recent run (for profiling)


def kernel(x, W1, b1, W2, b2):
    global LAST_RESULT
    in_maps = host_inputs(x, W1, b1, W2, b2)
    nc = bacc.Bacc("TRN2", target_bir_lowering=False, debug=False)
    build(nc)
    nc.compile()
    LAST_RESULT = run_bass_kernel_spmd(nc, in_maps, list(range(NCORES)))
    return assemble(LAST_RESULT.results)


# revision 24
# speedup vs baseline: 1.0606x; 1.0606x over previous
"""AudioSNN Trainium2 kernel (v4).

Two-layer leaky-integrate-and-fire SNN (snntorch Leaky, reset-by-subtract),
T=500 recurrent steps over batch 4096, data-parallel over 8 NeuronCores
(512 batch elements per core).

Math (per step t, reference):
    cur1 = x_t @ W1.T + b1
    m1   = beta*m1 + cur1 - H(m1_prev - 1)
    spk1 = H(m1 - 1)
    cur2 = spk1 @ W2.T + b2
    m2   = beta*m2 + cur2 - spk2[t-1]
    spk2 = H(m2 - 1)    -> output [T, B, 5]

Device formulation (per core; recurrent state fp32, matmuls fp16):
  cur1 is spike-exact (host-validated) via a 3-term fp16 hi/lo split packed
  into ONE K=122 matmul per batch half:
    x' = 16*x = xh + xl (fp16), w' = 16*W1.T = wh + wl (fp16)
    lhsT = [wh; bias_hi; wl; bias_lo; wh]  (122 x 128)
    rhs  = [xh; 16s;     xh; 16s;     xl]  -> p1 = K1*cur1', K1 = 256
  Layer-1 state z1 = K1*(m1 - 1).  Batch halves A (0:256) / B (256:512) run
  as independent recurrences in separate PSUM banks:
    p1_h += mask_h @ s_h[t-1]      (A: -0.5*K1*I @ sign; B: -K1*I @ spike)
    z1_h[t] = beta*z1_h[t-1] + p1_h        (DVE)
    s_A[t] = Sign(z1_A[t]) on ACT (+-1 fp16); s_B[t] = z1_B > 0 on DVE (0/1)
  Layer 2 column-grouped ([5,512] -> [128,128], batch quarter q at
  partitions 32q+o) and scaled by S2=64 (fp16-normal-range W2 split).
  y2 = S2*(m2 - 1 - p_q); quarters 0,1 consume sign-form (w2a = S2*0.5*W2.T,
  p from C2a = 0.5*sum(W2)+b2+beta-1), quarters 2,3 spike-form
  (w2b = S2*W2.T, C2b = b2+beta-1):
    p2[32q:32q+5,:] = w2{a,b}hi^T.T @ s_q + w2{a,b}lo^T.T @ s_q
    p2 += (-S2*Imask)_fp16 @ spk2'[t-1]
    y2[t] = beta*y2[t-1] + p2              (DVE)
    spk2'[t] = (y2[t] > -S2*p')            (DVE, fp16 0/1, DMA'd per chunk)
"""

import sys

sys.path.insert(0, "/opt/trn_rl_repo")

from contextlib import ExitStack

import numpy as np

from concourse import bacc, mybir, tile
from concourse.bass_utils import run_bass_kernel_spmd

BETA = 0.9
T, F, H, O = 500, 40, 128, 5
NCORES = 8
BC = 512  # batch per core
G2 = 256  # batch per half (layer-1 chain granularity)
CH = 20  # time steps per DMA chunk (must divide T)
S2 = 64.0  # layer-2 state scale
XS = 16.0  # x scale; W1 also scaled by XS; layer-1 state scale K1 = XS*XS
AHEAD = 2  # cur1 matmul emission lead (steps)
F32 = mybir.dt.float32
FP16 = mybir.dt.float16

MULT = mybir.AluOpType.mult
ADD = mybir.AluOpType.add
IS_GT = mybir.AluOpType.is_gt

KC = 3 * F + 2  # 122: [xh(40); ones; xh(40); ones; xl(40)]


def build(nc, n_steps=T, ch=CH):
    n_chunks = n_steps // ch

    x_d = nc.dram_tensor(
        "x_cat", [n_chunks, KC * ch * BC], FP16, kind="ExternalInput"
    ).ap()
    w1a_d = nc.dram_tensor("w1cat_a", [KC, H], FP16, kind="ExternalInput").ap()
    w1b_d = nc.dram_tensor("w1cat_b", [KC, H], FP16, kind="ExternalInput").ap()
    mska_d = nc.dram_tensor("mask_a", [H, H], FP16, kind="ExternalInput").ap()
    nim_d = nc.dram_tensor("neg_i_mask", [H, H], FP16, kind="ExternalInput").ap()
    w2_d = [
        nc.dram_tensor(nm, [H, O], FP16, kind="ExternalInput").ap()
        for nm in ["w2ahi", "w2alo", "w2bhi", "w2blo"]
    ]
    nth_d = nc.dram_tensor("nthr2", [H, 1], F32, kind="ExternalInput").ap()
    z2i_d = nc.dram_tensor("z2init", [H, H], F32, kind="ExternalInput").ap()
    out_d = nc.dram_tensor(
        "out", [n_chunks, 4 * O * ch * H], FP16, kind="ExternalOutput"
    ).ap()

    with tile.TileContext(nc) as tc, ExitStack() as ctx:
        const = ctx.enter_context(tc.tile_pool(name="const", bufs=1))
        state = ctx.enter_context(tc.tile_pool(name="state", bufs=1))
        xin = ctx.enter_context(tc.tile_pool(name="xin", bufs=3))
        outp = ctx.enter_context(tc.tile_pool(name="outp", bufs=3))
        ps1 = ctx.enter_context(tc.tile_pool(name="ps1", bufs=3, space="PSUM"))
        ps2 = ctx.enter_context(tc.tile_pool(name="ps2", bufs=2, space="PSUM"))

        w1a_s = const.tile([KC, H], FP16, tag="w1a")
        w1b_s = const.tile([KC, H], FP16, tag="w1b")
        mska_s = const.tile([H, H], FP16, tag="mska")
        nim_s = const.tile([H, H], FP16, tag="nim")
        w2_s = [
            const.tile([H, O], FP16, tag=f"w2_{i}", name=f"w2_{i}")
            for i in range(4)
        ]
        nth_s = const.tile([H, 1], F32, tag="nth")
        for s, d in [
            (w1a_s, w1a_d),
            (w1b_s, w1b_d),
            (mska_s, mska_d),
            (nim_s, nim_d),
            (nth_s, nth_d),
        ] + list(zip(w2_s, w2_d)):
            nc.sync.dma_start(out=s[:], in_=d[:])

        # Recurrent state, ping-pong buffered (index = t % 2).
        z1 = [state.tile([H, BC], F32, tag=f"z1_{p}", name=f"z1_{p}") for p in range(2)]
        g = [state.tile([H, BC], FP16, tag=f"g_{p}", name=f"g_{p}") for p in range(2)]
        z2 = [state.tile([H, H], F32, tag=f"z2_{p}", name=f"z2_{p}") for p in range(2)]
        spk0 = state.tile([H, H], FP16, tag="spk0")

        nc.vector.memset(z1[1][:], -XS * XS)  # m1(0)=0 -> z1 = -K1
        nc.vector.memset(g[1][:, 0:G2], -1.0)  # sign(-z)
        nc.vector.memset(g[1][:, G2:BC], 0.0)  # spike form
        nc.sync.dma_start(out=z2[1][:], in_=z2i_d[:])
        nc.vector.memset(spk0[:], -1.0)  # sign form: no spike

        xts = {}
        p1s = {}
        ot = None
        spk_prev = spk0[:]

        def fetch_x(chk):
            """Issue the chunk-chk x load, split across the three DMA
            issue paths (SWDGE + both HWDGE rings) so it streams at
            ~100 GB/s instead of a single ring's ~27."""
            xt = xin.tile([KC, ch * BC], FP16, tag="xt")
            rsp = [0, 70, 101, KC]
            for eng, (r0, r1) in zip(
                (nc.gpsimd, nc.scalar, nc.sync), zip(rsp[:-1], rsp[1:])
            ):
                eng.dma_start(
                    out=xt[r0:r1, :],
                    in_=x_d[chk : chk + 1, r0 * ch * BC : r1 * ch * BC],
                )
            xts[chk] = xt
            xts.pop(chk - 3, None)

        def emit_c1(tf):
            """Emit the cur1 matmuls for step tf (one per batch half)."""
            chk, st = divmod(tf, ch)
            xt = xts[chk]
            pair = []
            for hf in range(2):
                p1 = ps1.tile([H, G2], F32, tag=f"p1_{hf}")
                nc.tensor.matmul(
                    p1[:],
                    w1a_s[:] if hf == 0 else w1b_s[:],
                    xt[:, st * BC + hf * G2 : st * BC + (hf + 1) * G2],
                    start=True,
                    stop=False,
                )
                pair.append(p1)
            p1s[tf] = pair

        l2state = {"spk_prev": spk_prev, "ot": None}

        def emit_l2(tl):
            """Column-grouped layer 2 for step tl ([128,128], scale S2)."""
            chk, st = divmod(tl, ch)
            cur, prv = tl % 2, 1 - (tl % 2)
            if st == 0:
                l2state["ot"] = outp.tile([H, ch * H], FP16, tag="ot", name="ot")
            ot = l2state["ot"]
            p2 = ps2.tile([H, H], F32, tag="p2")
            nc.tensor.matmul(
                p2[:], nim_s[:], l2state["spk_prev"], start=True, stop=False
            )
            order = [(0, 0), (0, 1), (1, 0), (1, 1), (2, 2), (2, 3), (3, 2), (3, 3)]
            for i, (w, q) in enumerate(order):
                nc.tensor.matmul(
                    p2[32 * q : 32 * q + O, :],
                    w2_s[w],
                    g[cur][:, q * H : (q + 1) * H],
                    start=False,
                    stop=(i == 7),
                    tile_position=(0, 32 * q),
                )
            nc.vector.scalar_tensor_tensor(
                z2[cur][:], z2[prv][:], BETA, p2[:], MULT, ADD
            )
            o_slice = ot[:, st * H : (st + 1) * H]
            nc.scalar.sign(o_slice, z2[cur][:], bias=nth_s[:])
            l2state["spk_prev"] = o_slice
            if st == ch - 1:
                for q in range(4):
                    nc.sync.dma_start(
                        out=out_d[chk : chk + 1, q * O * ch * H : (q + 1) * O * ch * H],
                        in_=ot[32 * q : 32 * q + O, :],
                    )

        fetch_x(0)
        for tf in range(min(AHEAD, n_steps)):
            emit_c1(tf)

        for t in range(n_steps):
            chk, st = divmod(t, ch)
            if st == 1 and chk + 1 < n_chunks:
                fetch_x(chk + 1)  # prefetch a full chunk ahead
            cur, prv = t % 2, 1 - (t % 2)

            # ---- layer 1, independent batch halves ----
            for hf, msk in ((0, mska_s), (1, mska_s)):
                bsl = slice(hf * G2, (hf + 1) * G2)
                p1 = p1s[t][hf]
                nc.tensor.matmul(
                    p1[:], msk[:], g[prv][:, bsl], start=False, stop=True
                )
                nc.vector.scalar_tensor_tensor(
                    z1[cur][:, bsl], z1[prv][:, bsl], BETA, p1[:], MULT, ADD
                )
                if hf == 0:
                    nc.scalar.sign(g[cur][:, bsl], z1[cur][:, bsl])
                else:
                    # {0,2}-valued spikes: same -0.5*K1*I fold mask as the
                    # sign-form half (w2 for these quarters is halved)
                    nc.vector.tensor_scalar(
                        g[cur][:, bsl], z1[cur][:, bsl], 0.0, 2.0, IS_GT, MULT
                    )
            del p1s[t]
            # emit the step-(t+AHEAD) cur1 matmuls HERE: they sit between the
            # folds and the rest of the PE FIFO, streaming while this step's
            # spikes are still in flight
            if t + AHEAD < n_steps:
                emit_c1(t + AHEAD)
            # layer 2 runs one step BEHIND layer 1: every op in emit_l2(t-1)
            # has its inputs ready, so neither the PE nor the DVE FIFO ever
            # blocks the layer-1 chain on layer-2 work
            if t > 0:
                emit_l2(t - 1)
        emit_l2(n_steps - 1)


def _split16(a):
    hi = a.astype(np.float16)
    lo = (a - hi.astype(np.float32)).astype(np.float16)
    return hi, lo


def host_inputs(x, W1, b1, W2, b2, n_steps=T, ch=CH):
    """Shard + precompute all per-core device input arrays."""
    n_chunks = n_steps // ch
    x = np.asarray(x, np.float32)[:, :n_steps, :]
    W1 = np.asarray(W1, np.float32)
    b1 = np.asarray(b1, np.float32)
    W2 = np.asarray(W2, np.float32)
    b2 = np.asarray(b2, np.float32)

    # x*XS split into fp16 hi+lo; rows: [xh(40); 16s; xh(40); 16s; xl(40)]
    xs = x.reshape(NCORES, BC, n_steps, F).transpose(0, 2, 3, 1)  # [8,T',40,512]
    xh = (XS * xs).astype(np.float16)
    xl = (XS * xs - xh.astype(np.float32)).astype(np.float16)
    cat = np.empty((NCORES, n_steps, KC, BC), np.float16)
    cat[:, :, 0:F, :] = xh
    cat[:, :, F, :] = np.float16(XS)
    cat[:, :, F + 1 : 2 * F + 1, :] = xh
    cat[:, :, 2 * F + 1, :] = np.float16(XS)
    cat[:, :, 2 * F + 2 :, :] = xl
    cat = cat.reshape(NCORES, n_chunks, ch, KC, BC).transpose(0, 1, 3, 2, 4)
    x_cat = np.ascontiguousarray(cat).reshape(NCORES, n_chunks, KC * ch * BC)

    # lhsT rows: [wh(40); bias_hi; wl(40); bias_lo; wh(40)]
    # half A (sign-form reset): bias b1 + beta - 1.5; half B (spike-form
    # reset): bias b1 + beta - 1.0
    w1t = W1.T * XS  # [40, 128]
    w1hi, w1lo = _split16(w1t)

    def w1cat_for(bias_shift):
        b1p = (b1 + BETA - bias_shift) * XS
        bhi, blo = _split16(b1p[None, :])
        return np.concatenate([w1hi, bhi, w1lo, blo, w1hi], axis=0)  # [122,128]

    w1cat_a = w1cat_for(1.5)
    w1cat_b = w1cat_for(1.0)

    K1 = XS * XS
    mask_a = (-0.5 * K1 * np.eye(H)).astype(np.float16)  # both halves
    used = np.zeros(H, np.float32)
    for q in range(4):
        used[32 * q : 32 * q + O] = 1.0
    # output spikes are in sign form: -S2*spk = -0.5*S2*s - 0.5*S2, the
    # constant part is absorbed into a state shift of 0.5*S2/(1-beta)
    neg_i_mask = (-0.5 * S2 * np.diag(used)).astype(np.float16)

    w2a = (S2 * 0.5 * W2.T).astype(np.float32)  # sign-form quarters
    w2b = (S2 * 0.5 * W2.T).astype(np.float32)  # {0,2}-spike quarters
    w2ahi, w2alo = _split16(w2a)
    w2bhi, w2blo = _split16(w2b)

    C2a = 0.5 * W2.sum(axis=1) + b2 + BETA - 1.0
    C2b = b2 + BETA - 1.0
    pa = (C2a / (1.0 - BETA)).astype(np.float32)
    pb = (C2b / (1.0 - BETA)).astype(np.float32)
    shift = 0.5 * S2 / (1.0 - BETA)  # 320
    nthr2 = np.zeros((H, 1), np.float32)
    z2init = np.zeros((H, H), np.float32)
    for q in range(4):
        p = pa if q < 2 else pb
        nthr2[32 * q : 32 * q + O, 0] = -(shift - S2 * p)
        z2init[32 * q : 32 * q + O, :] = (S2 * (-1.0 - p) + shift)[:, None]

    shared = {
        "w1cat_a": w1cat_a,
        "w1cat_b": w1cat_b,
        "mask_a": mask_a,
        "neg_i_mask": neg_i_mask,
        "w2ahi": w2ahi,
        "w2alo": w2alo,
        "w2bhi": w2bhi,
        "w2blo": w2blo,
        "nthr2": nthr2,
        "z2init": z2init,
    }
    return [{"x_cat": x_cat[c], **shared} for c in range(NCORES)]


def assemble(results, n_steps=T, ch=CH):
    """per-core out [n_chunks, 4*5*ch*128] fp16 -> [T', B, O] float32."""
    n_chunks = n_steps // ch
    outs = []
    for r in results:
        a = np.asarray(r["out"]).astype(np.float32)
        a = (a + 1.0) * 0.5  # sign form -> 0/1
        a = a.reshape(n_chunks, 4, O, ch, H)  # [k, q, o, st, b]
        a = a.transpose(0, 3, 1, 4, 2).reshape(n_steps, BC, O)
        outs.append(a)
    return np.concatenate(outs, axis=1)


LAST_RESULT = None  # BassKernelResults of the most recent run (for profiling)


def kernel(x, W1, b1, W2, b2):
    global LAST_RESULT
    in_maps = host_inputs(x, W1, b1, W2, b2)
    nc = bacc.Bacc("TRN2", target_bir_lowering=False, debug=False)
    build(nc)
    nc.compile()
    LAST_RESULT = run_bass_kernel_spmd(nc, in_maps, list(range(NCORES)))
    return assemble(LAST_RESULT.results)
